# revision 18
# baseline (speedup 1.0000x reference)
"""Bahdanau additive attention on TRN2, data-parallel over batch on 8 NeuronCores.

Reference computation (per batch b):
    pre[s, :]  = W1 @ hs[s, b, :] + b1 + W2 @ hidden[b, :] + b2      # (S, H)
    energy[s]  = v . tanh(pre[s, :])                                  # (S,)
    energy     = where(mask[s, b], energy, -1e10)
    attn       = softmax(energy over s)
    ctx[b, :]  = sum_s attn[s] * hs[s, b, :]                          # (H,)

Key optimizations over a dense kernel:
  - Mask compaction on the host: masked-out s positions contribute exactly
    zero attention (energy -1e10 -> exp 0), so only the unmasked positions
    are shipped/processed. Per-batch sequences are gathered to
    NP = roundup(max count, 512); pad columns carry mask=1.
  - fp16 matmul inputs (true 1 cycle/row on the PE; f32r measures ~1.3),
    fp32 PSUM accumulation. Measured max-rel-err ~2.4e-3 (gate 2e-2).
  - SBUF layouts keep every matmul's moving-data reads CONTIGUOUS: strided
    rhs jumps between back-to-back matmuls cost ~+50ns each (measured), so
    hst is stored [(c,k) blocks, 512] and tanh outputs go to a per-block
    ring [m*512] so the k/m loops stream sequentially.
  - q = W2 @ hidden + b1 + b2 is computed on the host (0.02% of FLOPs,
    S-independent bias prep) and uploaded as the tanh per-partition bias.
  - Context for batches 0..BL-2: exp weights row is broadcast to all 128
    partitions by one PE rank-1 matmul (ones x em16) into PSUM, then
    ctx[h-chunk] = sum_s hst[h,s]*w[s] is a DVE scalar_tensor_tensor
    free-axis accumulate per 128-row h-chunk (no second hs copy, no PE).
  - Context for the LAST batch runs on the then-idle PE instead (s-major
    hs copy + transposed-exp weights + 16 M=1 matmuls), cutting the
    end-of-kernel serial tail roughly in half.
  - Softmax is unnormalized on device; Z (or its per-partition partials)
    is exported and divided out on the host during unsharding.
"""

import sys
from contextlib import ExitStack

import numpy as np

# Fallback path for concourse; the axon sitecustomize normally provides it.
if "/opt/trn_rl_repo" not in sys.path:
    sys.path.append("/opt/trn_rl_repo")

import concourse.bass as bass
import concourse.bacc as bacc
import concourse.mybir as mybir
import concourse.tile as tile
from concourse import bass_utils

S, B, H = 2048, 32, 1024
NCORES = 8
BL = B // NCORES  # local batches per core
HK = H // 128     # 128-partition chunks of H

F32 = mybir.dt.float32
F32R = mybir.dt.float32r
FP16 = mybir.dt.float16
U8 = mybir.dt.uint8
AF = mybir.ActivationFunctionType
AX = mybir.AxisListType

_CACHE = {}


def _emit(tc, aps, NP):
    nc = tc.nc
    ctx = aps["ctx_stack"]
    C = NP // 512   # 512-wide sigma blocks
    TP = NP // 128  # 128-wide chunks (last-batch transposes / hsn tiles)
    hst, w1m, qt, vt, cst, masku, hsn = (
        aps["hst"], aps["w1m"], aps["qt"], aps["vt"], aps["cst"],
        aps["masku"], aps["hsn"],
    )
    ctxT_out, ctxr_out, zs_out, zsp_out = (
        aps["ctxT"], aps["ctxr"], aps["zs"], aps["zsp"],
    )

    def pool(name, bufs, space="SBUF"):
        return ctx.enter_context(tc.tile_pool(name=name, bufs=bufs, space=space))

    p_hst = pool("hst", 3)
    p_w1 = pool("w1", 1)
    p_small = pool("small", 1)
    p_mask = pool("mask", 1)
    p_th = pool("th", 2)
    p_em32 = pool("em32", 2)
    p_em16 = pool("em16", 2)
    p_emt = pool("emt", 1)
    p_scr = pool("scr", 2)
    p_ctxT = pool("ctxT", 2)
    p_sc = pool("sc", 8)
    p_nm = pool("nm", 1)
    p_hsn = pool("hsn", 1)

    pp_pre = pool("ppre", 3, space="PSUM")
    pp_en = pool("pen", 2, space="PSUM")
    pp_b = pool("pb", 1, space="PSUM")
    pp_t = pool("pt", 1, space="PSUM")

    # ---------------- setup DMAs ----------------
    # w1 m=0 chunk first (unblocks the very first matmuls), rest behind.
    w1_sb = p_w1.tile([128, HK * HK * 128], FP16, tag="w1")
    hst_t = {}

    def load_hst(b, queue, split=False):
        t = p_hst.tile([128, C * HK * 512], FP16, tag="hst", name=f"hst{b}")
        if split:
            for c in range(C):
                queue.dma_start(
                    t[:, c * HK * 512:(c + 1) * HK * 512],
                    hst[b, :, c * HK * 512:(c + 1) * HK * 512],
                )
        else:
            queue.dma_start(t[:], hst[b])
        hst_t[b] = t

    # priority order on one queue: the DMA engines drain a queue roughly
    # in order, so startup-critical bytes must precede prefetches.
    t0 = p_hst.tile([128, C * HK * 512], FP16, tag="hst", name="hst0")
    nc.sync.dma_start(t0[:, 0:HK * 128], hst[0, :, 0:HK * 128])
    nc.sync.dma_start(w1_sb[:, 0:HK * 128], w1m[:, 0:HK * 128])
    nc.sync.dma_start(t0[:, HK * 128:HK * 256], hst[0, :, HK * 128:HK * 256])
    nc.sync.dma_start(t0[:, HK * 256:HK * 512], hst[0, :, HK * 256:HK * 512])
    hst_t[0] = t0
    nc.sync.dma_start(w1_sb[:, HK * 128:2 * HK * 128], w1m[:, HK * 128:2 * HK * 128])
    nc.sync.dma_start(w1_sb[:, 2 * HK * 128:4 * HK * 128], w1m[:, 2 * HK * 128:4 * HK * 128])
    nc.sync.dma_start(w1_sb[:, 4 * HK * 128:], w1m[:, 4 * HK * 128:])
    for c in range(1, C):
        nc.sync.dma_start(t0[:, c * HK * 512:(c + 1) * HK * 512],
                          hst[0, :, c * HK * 512:(c + 1) * HK * 512])

    # small constants on the vector queue (parallel issue)
    qt_sb = p_small.tile([128, BL * HK], F32, tag="qt")
    nc.scalar.dma_start(qt_sb[:], qt[:])
    vt_sb = p_small.tile([128, HK], FP16, tag="vt")
    nc.scalar.dma_start(vt_sb[:], vt[:])
    cst_sb = p_small.tile([1, 130], FP16, tag="cst")
    nc.scalar.dma_start(cst_sb[:], cst[:])
    ones16 = cst_sb[:, 0:128]
    ident32 = cst_sb[:, 128:130].bitcast(F32)
    mask_all = p_mask.tile([1, BL * NP], U8, tag="mask")
    nc.scalar.dma_start(mask_all[:], masku[:])

    em32_t = {}
    em16_t = {}

    # ------------- pass 1: energies for one (batch, sigma-block) -------------
    def p1_block(b, c):
        if c == 0:
            em32_t[b] = p_em32.tile([1, NP], F32, tag="em32", name=f"em32_{b}")
        hst_c = hst_t[b]
        pen = pp_en.tile([1, 512], F32, tag="pen", name=f"pen_{b}_{c}")
        thr = p_th.tile([128, HK * 512], FP16, tag="th", name=f"th_{b}_{c}")
        for m in range(HK):
            ppre = pp_pre.tile([128, 512], F32, tag="ppre", name=f"ppre_{b}_{c}_{m}")
            for k in range(HK):
                nc.tensor.matmul(
                    ppre[:],
                    lhsT=w1_sb[:, (m * HK + k) * 128:(m * HK + k + 1) * 128],
                    rhs=hst_c[:, (c * HK + k) * 512:(c * HK + k + 1) * 512],
                    start=(k == 0), stop=(k == HK - 1),
                )
            nc.scalar.activation(
                thr[:, m * 512:(m + 1) * 512], ppre[:], AF.Tanh,
                bias=qt_sb[:, BL * m + b:BL * m + b + 1], scale=1.0,
            )
        # energy matmuls as one sequential run over the thr ring; on-PE v-dot
        # reads th exactly once with no extra SBUF traffic (a DVE FMA tree
        # measured SLOWER overall: its acc read+write traffic contends with
        # the PE rhs stream and tanh writes, slowing both by ~20%).
        for m in range(HK):
            nc.tensor.matmul(
                pen[:], lhsT=vt_sb[:, m:m + 1],
                rhs=thr[:, m * 512:(m + 1) * 512],
                start=(m == 0), stop=(m == HK - 1),
            )
        # mask + PSUM drain in one DVE op: em = minv * -1e10 + energy
        nc.vector.scalar_tensor_tensor(
            em32_t[b][:, 512 * c:512 * (c + 1)],
            mask_all[:, b * NP + 512 * c:b * NP + 512 * (c + 1)],
            -1e10, pen[:],
            op0=mybir.AluOpType.mult, op1=mybir.AluOpType.add,
        )

    # ------------- softmax row path (batches 0..BL-2) -------------
    def sm_row(b):
        em32 = em32_t.pop(b)
        negmax = p_sc.tile([1, 1], F32, tag="negmax", name=f"negmax{b}")
        nc.vector.reduce_max(negmax[:], em32[:], axis=AX.X, negate=True)
        em16 = p_em16.tile([1, NP], FP16, tag="em16", name=f"em16_{b}")
        zs = p_sc.tile([1, 1], F32, tag="zs", name=f"zs{b}")
        nc.scalar.activation(
            em16[:], em32[:], AF.Exp, bias=negmax[:], scale=1.0, accum_out=zs[:]
        )
        nc.gpsimd.dma_start(zs_out[b:b + 1, :], zs[:])
        em16_t[b] = em16

    pb_t = {}

    # ------------- pass 2 for batches 0..BL-2: DVE free-axis reduce -------
    def p2_bcast(b):
        # broadcast the weights row to all partitions: rank-1 PE matmul.
        # Emitted right after sm_row so the DVE ctx accumulates can overlap
        # the NEXT batch's matmul blocks instead of queueing behind them.
        em16 = em16_t.pop(b)
        pb = pp_b.tile([128, NP], F32, tag="pb", name=f"pb_{b}")
        for c in range(C):
            nc.tensor.matmul(
                pb[:, 512 * c:512 * (c + 1)],
                lhsT=ones16,
                rhs=em16[:, 512 * c:512 * (c + 1)],
                start=True, stop=True,
            )
        pb_t[b] = pb

    def p2_dve(b):
        hst_c = hst_t.pop(b)
        pb = pb_t.pop(b)
        ctxT = p_ctxT.tile([128, HK], F32, tag="ctxT", name=f"ctxT_{b}")
        hview = hst_c[:].rearrange("p (c k f) -> p c k f", c=C, k=HK, f=512)
        bview = pb[:].rearrange("p (c f) -> p c f", c=C, f=512)
        for m in range(HK):
            scr = p_scr.tile([128, NP], FP16, tag="scr", name=f"scr_{b}_{m}")
            nc.vector.scalar_tensor_tensor(
                out=scr[:].rearrange("p (c f) -> p c f", c=C, f=512),
                in0=hview[:, :, m, :],
                scalar=1.0,
                in1=bview,
                op0=mybir.AluOpType.mult, op1=mybir.AluOpType.mult,
                accum_out=ctxT[:, m:m + 1],
            )
        nc.gpsimd.dma_start(ctxT_out[b], ctxT[:])

    # ------------- pass 2 for the last batch: PE path -------------
    def p2_pe(b):
        em32 = em32_t.pop(b)
        hst_t.pop(b)
        # transpose energies to [s%128 partition, s//128]; the global max is
        # then a cheap per-partition max + gpsimd cross-partition all-reduce
        # (the [1,NP] row max would run on a single DVE lane).
        pt = pp_t.tile([128, TP], F32, tag="pt", name="ptT")
        for t in range(TP):
            nc.tensor.transpose(
                pt[:, t:t + 1], em32[:, 128 * t:128 * (t + 1)], ident32
            )
        pmax = p_sc.tile([128, 1], F32, tag="pmax")
        nc.vector.reduce_max(pmax[:], pt[:], axis=AX.X)
        gmax = p_sc.tile([128, 1], F32, tag="gmax")
        import concourse.bass_isa as bass_isa
        nc.gpsimd.partition_all_reduce(gmax[:], pmax[:], channels=128,
                                       reduce_op=bass_isa.ReduceOp.max)
        nmb = p_nm.tile([128, 1], F32, tag="nmb")
        nc.scalar.mul(nmb[:], gmax[:], -1.0)
        emt = p_emt.tile([128, TP], FP16, tag="emt")
        zsp = p_sc.tile([128, 1], F32, tag="zsp")
        nc.scalar.activation(
            emt[:], pt[:], AF.Exp, bias=nmb[:], scale=1.0, accum_out=zsp[:]
        )
        nc.gpsimd.dma_start(zsp_out[:], zsp[:])
        hsn_c = hsn_t[0]
        pc = [
            pp_en.tile([1, 512], F32, tag="pen", name=f"pctx{n}")
            for n in range(2)
        ]
        ctxr_sb = p_emt.tile([1, H], F32, tag="ctxr")
        for n in range(2):
            for t in range(TP):
                nc.tensor.matmul(
                    pc[n][:],
                    lhsT=emt[:, t:t + 1],
                    rhs=hsn_c[:, (n * TP + t) * 512:(n * TP + t + 1) * 512],
                    start=(t == 0), stop=(t == TP - 1),
                )
            nc.vector.tensor_copy(ctxr_sb[:, 512 * n:512 * (n + 1)], pc[n][:])
            nc.gpsimd.dma_start(ctxr_out[:, 512 * n:512 * (n + 1)],
                                ctxr_sb[:, 512 * n:512 * (n + 1)])

    hsn_t = {}

    def load_hsn():
        t = p_hsn.tile([128, TP * H], FP16, tag="hsn")
        nc.sync.dma_start(t[:], hsn[:])
        hsn_t[0] = t

    # ------------- schedule -------------
    if BL > 1:
        load_hst(1, nc.sync)
    for c in range(C):
        p1_block(0, c)
    for b in range(1, BL):
        if b + 1 < BL:
            load_hst(b + 1, nc.sync)
        if b == min(2, BL - 1):
            load_hsn()
        p1_block(b, 0)
        if b - 1 < BL - 1:
            sm_row(b - 1)
            p2_bcast(b - 1)
        for c in range(1, C):
            p1_block(b, c)
        p2_dve(b - 1)
    if BL == 1:
        load_hsn()
    p2_pe(BL - 1)


def build_program(NP=1024):
    key = ("nc", NP)
    if key in _CACHE:
        return _CACHE[key]
    C = NP // 512
    TP = NP // 128
    nc = bacc.Bacc("TRN2", target_bir_lowering=False, debug=False, enable_asserts=False)
    aps = {
        "hst": nc.dram_tensor("hst", (BL, 128, C * HK * 512), FP16, kind="ExternalInput").ap(),
        "w1m": nc.dram_tensor("w1m", (128, HK * HK * 128), FP16, kind="ExternalInput").ap(),
        "qt": nc.dram_tensor("qt", (128, BL * HK), F32, kind="ExternalInput").ap(),
        "vt": nc.dram_tensor("vt", (128, HK), FP16, kind="ExternalInput").ap(),
        "cst": nc.dram_tensor("cst", (1, 130), FP16, kind="ExternalInput").ap(),
        "masku": nc.dram_tensor("masku", (1, BL * NP), U8, kind="ExternalInput").ap(),
        "hsn": nc.dram_tensor("hsn", (128, TP * H), FP16, kind="ExternalInput").ap(),
        "ctxT": nc.dram_tensor("ctxT", (BL, 128, HK), F32, kind="ExternalOutput").ap(),
        "ctxr": nc.dram_tensor("ctxr", (1, H), F32, kind="ExternalOutput").ap(),
        "zs": nc.dram_tensor("zs", (BL, 1), F32, kind="ExternalOutput").ap(),
        "zsp": nc.dram_tensor("zsp", (128, 1), F32, kind="ExternalOutput").ap(),
    }
    with tile.TileContext(nc) as tc:
        with ExitStack() as stack:
            aps["ctx_stack"] = stack
            _emit(tc, aps, NP)
    nc.compile()
    _CACHE[key] = nc
    return nc


def prep_in_maps(inputs):
    hs = np.asarray(inputs["hidden_sequence"], dtype=np.float32)
    hid = np.asarray(inputs["hidden"], dtype=np.float32)[0]  # (B, H)
    masks = np.asarray(inputs["input_masks"]).astype(bool)
    W1 = np.asarray(inputs["W1"], dtype=np.float32)
    W2 = np.asarray(inputs["W2"], dtype=np.float32)
    b1 = np.asarray(inputs["b1"], dtype=np.float32)
    b2 = np.asarray(inputs["b2"], dtype=np.float32)
    v = np.asarray(inputs["v"], dtype=np.float32)

    counts = masks.sum(axis=0)
    NP = max(512, int(-(-int(counts.max()) // 512)) * 512)
    C = NP // 512
    TP = NP // 128

    # w1m[p, (m*HK + k)*128 + j] = W1[128m + j, 128k + p]
    w1m = np.ascontiguousarray(
        W1.reshape(HK, 128, HK, 128).transpose(3, 0, 2, 1).reshape(128, HK * HK * 128)
    ).astype(np.float16)
    vt = np.ascontiguousarray(v.reshape(HK, 128).T).astype(np.float16)
    cst = np.zeros((1, 130), dtype=np.float16)
    cst[0, :128] = 1.0
    cst[0, 128:130] = np.frombuffer(
        np.float32(1.0).tobytes(), dtype=np.float16
    )
    # q[b, :] = W2 @ hidden[b] + b1 + b2 (host bias prep, S-independent)
    qfull = (hid.astype(np.float16).astype(np.float32)
             @ W2.astype(np.float16).astype(np.float32).T + b1 + b2)  # (B, H)

    in_maps = []
    for ci in range(NCORES):
        hstp = np.zeros((BL, 128, C * HK * 512), dtype=np.float16)
        hsnp = np.zeros((128, TP * H), dtype=np.float16)
        invm = np.ones((BL, NP), dtype=np.uint8)
        for bi in range(BL):
            b = BL * ci + bi
            idx = np.flatnonzero(masks[:, b])
            n = len(idx)
            hb = np.zeros((NP, H), dtype=np.float16)
            hb[:n] = hs[idx, b, :].astype(np.float16)  # compact (n, H)
            # hst[b, p, (c*HK + k)*512 + j] = hb[512c + j, 128k + p]
            hstp[bi] = (
                hb.reshape(C, 512, HK, 128).transpose(3, 0, 2, 1).reshape(128, C * HK * 512)
            )
            if bi == BL - 1:
                # hsn[p, (n*TP + t)*512 + j] = hb[128t + p, 512n + j]: the
                # last batch's ctx matmuls stream rhs contiguously in t.
                hsnp[:] = (
                    hb.reshape(TP, 128, 2, 512).transpose(1, 2, 0, 3)
                    .reshape(128, TP * H)
                )
            invm[bi, :n] = 0
        g = slice(BL * ci, BL * (ci + 1))
        # qt[p, BL*m + b] = q[b, 128m + p]
        qtp = np.ascontiguousarray(
            qfull[g].T.reshape(HK, 128, BL).transpose(1, 0, 2).reshape(128, HK * BL)
        )
        in_maps.append({
            "hst": hstp,
            "w1m": w1m,
            "qt": qtp,
            "vt": vt,
            "cst": cst,
            "masku": np.ascontiguousarray(invm.reshape(1, BL * NP)),
            "hsn": hsnp,
        })
    return in_maps, NP


def postprocess(results):
    """results[ci] -> dict with ctxT/ctxr/zs/zsp; returns (1,B,H) float32."""
    ctx = np.empty((B, H), dtype=np.float32)
    for ci in range(NCORES):
        r = results[ci]
        ctxT = np.asarray(r["ctxT"], dtype=np.float32)
        zs = np.asarray(r["zs"], dtype=np.float32)
        for bi in range(BL - 1):
            ctx[BL * ci + bi] = ctxT[bi].T.reshape(H) / zs[bi, 0]
        z_last = np.asarray(r["zsp"], dtype=np.float32).sum()
        ctx[BL * ci + BL - 1] = np.asarray(r["ctxr"], dtype=np.float32)[0] / z_last
    return ctx[None]


def kernel(**inputs):
    in_maps, NP = prep_in_maps(inputs)
    nc = build_program(NP)
    res = bass_utils.run_bass_kernel_spmd(nc, in_maps, list(range(NCORES)))
    return postprocess(res.results)


if __name__ == "__main__":
    build_program()
    print("program built OK")


# revision 19
# speedup vs baseline: 1.0308x; 1.0308x over previous
"""Bahdanau additive attention on TRN2, data-parallel over batch on 8 NeuronCores.

Reference computation (per batch b):
    pre[s, :]  = W1 @ hs[s, b, :] + b1 + W2 @ hidden[b, :] + b2      # (S, H)
    energy[s]  = v . tanh(pre[s, :])                                  # (S,)
    energy     = where(mask[s, b], energy, -1e10)
    attn       = softmax(energy over s)
    ctx[b, :]  = sum_s attn[s] * hs[s, b, :]                          # (H,)

Key optimizations over a dense kernel:
  - Mask compaction on the host: masked-out s positions contribute exactly
    zero attention (energy -1e10 -> exp 0), so only the unmasked positions
    are shipped/processed. Per-batch sequences are gathered to
    NP = roundup(max count, 512); pad columns carry mask=1.
  - fp16 matmul inputs (true 1 cycle/row on the PE; f32r measures ~1.3),
    fp32 PSUM accumulation. Measured max-rel-err ~2.4e-3 (gate 2e-2).
  - SBUF layouts keep every matmul's moving-data reads CONTIGUOUS: strided
    rhs jumps between back-to-back matmuls cost ~+50ns each (measured), so
    hst is stored [(c,k) blocks, 512] and tanh outputs go to a per-block
    ring [m*512] so the k/m loops stream sequentially.
  - q = W2 @ hidden + b1 + b2 is computed on the host (0.02% of FLOPs,
    S-independent bias prep) and uploaded as the tanh per-partition bias.
  - Context for batches 0..BL-2: exp weights row is broadcast to all 128
    partitions by one PE rank-1 matmul (ones x em16) into PSUM, then
    ctx[h-chunk] = sum_s hst[h,s]*w[s] is a DVE scalar_tensor_tensor
    free-axis accumulate per 128-row h-chunk (no second hs copy, no PE).
  - Context for the LAST batch runs on the then-idle PE instead (s-major
    hs copy + transposed-exp weights + 16 M=1 matmuls), cutting the
    end-of-kernel serial tail roughly in half.
  - Softmax is unnormalized on device; Z (or its per-partition partials)
    is exported and divided out on the host during unsharding.
"""

import sys
from contextlib import ExitStack

import numpy as np

# Fallback path for concourse; the axon sitecustomize normally provides it.
if "/opt/trn_rl_repo" not in sys.path:
    sys.path.append("/opt/trn_rl_repo")

import concourse.bass as bass
import concourse.bacc as bacc
import concourse.mybir as mybir
import concourse.tile as tile
from concourse import bass_utils

S, B, H = 2048, 32, 1024
NCORES = 8
BL = B // NCORES  # local batches per core
HK = H // 128     # 128-partition chunks of H

F32 = mybir.dt.float32
F32R = mybir.dt.float32r
FP16 = mybir.dt.float16
U8 = mybir.dt.uint8
AF = mybir.ActivationFunctionType
AX = mybir.AxisListType

_CACHE = {}


def _emit(tc, aps, NP):
    nc = tc.nc
    ctx = aps["ctx_stack"]
    C = NP // 512   # 512-wide sigma blocks
    TP = NP // 128  # 128-wide chunks (last-batch transposes / hsn tiles)
    hst, w1m, qt, vt, cst, masku, hsn = (
        aps["hst"], aps["w1m"], aps["qt"], aps["vt"], aps["cst"],
        aps["masku"], aps["hsn"],
    )
    ctxT_out, ctxr_out, zs_out, zsp_out = (
        aps["ctxT"], aps["ctxr"], aps["zs"], aps["zsp"],
    )

    def pool(name, bufs, space="SBUF"):
        return ctx.enter_context(tc.tile_pool(name=name, bufs=bufs, space=space))

    p_hst = pool("hst", 3)
    p_w1 = pool("w1", 1)
    p_small = pool("small", 1)
    p_mask = pool("mask", 1)
    p_th = pool("th", 2)
    p_em32 = pool("em32", 2)
    p_em16 = pool("em16", 2)
    p_emt = pool("emt", 1)
    p_scr = pool("scr", 2)
    p_ctxT = pool("ctxT", 2)
    p_sc = pool("sc", 8)
    p_nm = pool("nm", 1)
    p_hsn = pool("hsn", 1)

    pp_pre = pool("ppre", 3, space="PSUM")
    pp_en = pool("pen", 2, space="PSUM")
    pp_b = pool("pb", 1, space="PSUM")
    pp_t = pool("pt", 1, space="PSUM")

    # ---------------- setup DMAs ----------------
    # w1 m=0 chunk first (unblocks the very first matmuls), rest behind.
    w1_sb = p_w1.tile([128, HK * HK * 128], FP16, tag="w1")
    hst_t = {}

    def load_hst(b, queue, split=False):
        t = p_hst.tile([128, C * HK * 512], FP16, tag="hst", name=f"hst{b}")
        if split:
            for c in range(C):
                queue.dma_start(
                    t[:, c * HK * 512:(c + 1) * HK * 512],
                    hst[b, :, c * HK * 512:(c + 1) * HK * 512],
                )
        else:
            queue.dma_start(t[:], hst[b])
        hst_t[b] = t

    # priority order on one queue: the DMA engines drain a queue roughly
    # in order, so startup-critical bytes must precede prefetches.
    t0 = p_hst.tile([128, C * HK * 512], FP16, tag="hst", name="hst0")
    nc.sync.dma_start(t0[:, 0:HK * 128], hst[0, :, 0:HK * 128])
    nc.sync.dma_start(w1_sb[:, 0:HK * 128], w1m[:, 0:HK * 128])
    nc.sync.dma_start(t0[:, HK * 128:HK * 256], hst[0, :, HK * 128:HK * 256])
    nc.sync.dma_start(t0[:, HK * 256:HK * 512], hst[0, :, HK * 256:HK * 512])
    hst_t[0] = t0
    nc.sync.dma_start(w1_sb[:, HK * 128:2 * HK * 128], w1m[:, HK * 128:2 * HK * 128])
    nc.sync.dma_start(w1_sb[:, 2 * HK * 128:4 * HK * 128], w1m[:, 2 * HK * 128:4 * HK * 128])
    nc.sync.dma_start(w1_sb[:, 4 * HK * 128:], w1m[:, 4 * HK * 128:])
    for c in range(1, C):
        nc.sync.dma_start(t0[:, c * HK * 512:(c + 1) * HK * 512],
                          hst[0, :, c * HK * 512:(c + 1) * HK * 512])

    # small constants on the vector queue (parallel issue)
    qt_sb = p_small.tile([128, BL * HK], F32, tag="qt")
    nc.scalar.dma_start(qt_sb[:], qt[:])
    vt_sb = p_small.tile([128, HK], FP16, tag="vt")
    nc.scalar.dma_start(vt_sb[:], vt[:])
    cst_sb = p_small.tile([1, 130], FP16, tag="cst")
    nc.scalar.dma_start(cst_sb[:], cst[:])
    ones16 = cst_sb[:, 0:128]
    ident32 = cst_sb[:, 128:130].bitcast(F32)
    mask_all = p_mask.tile([1, BL * NP], U8, tag="mask")
    nc.scalar.dma_start(mask_all[:], masku[:])

    em32_t = {}
    em16_t = {}

    # ------------- pass 1: energies for one (batch, sigma-block) -------------
    def p1_block(b, c):
        if c == 0:
            em32_t[b] = p_em32.tile([1, NP], F32, tag="em32", name=f"em32_{b}")
        hst_c = hst_t[b]
        pen = pp_en.tile([1, 512], F32, tag="pen", name=f"pen_{b}_{c}")
        thr = p_th.tile([128, HK * 512], FP16, tag="th", name=f"th_{b}_{c}")
        for m in range(HK):
            ppre = pp_pre.tile([128, 512], F32, tag="ppre", name=f"ppre_{b}_{c}_{m}")
            for k in range(HK):
                nc.tensor.matmul(
                    ppre[:],
                    lhsT=w1_sb[:, (m * HK + k) * 128:(m * HK + k + 1) * 128],
                    rhs=hst_c[:, (c * HK + k) * 512:(c * HK + k + 1) * 512],
                    start=(k == 0), stop=(k == HK - 1),
                )
            nc.scalar.activation(
                thr[:, m * 512:(m + 1) * 512], ppre[:], AF.Tanh,
                bias=qt_sb[:, BL * m + b:BL * m + b + 1], scale=1.0,
            )
        # energy matmuls as one sequential run over the thr ring; on-PE v-dot
        # reads th exactly once with no extra SBUF traffic (a DVE FMA tree
        # measured SLOWER overall: its acc read+write traffic contends with
        # the PE rhs stream and tanh writes, slowing both by ~20%).
        for m in range(HK):
            nc.tensor.matmul(
                pen[:], lhsT=vt_sb[:, m:m + 1],
                rhs=thr[:, m * 512:(m + 1) * 512],
                start=(m == 0), stop=(m == HK - 1),
            )
        # mask + PSUM drain in one DVE op: em = minv * -1e10 + energy
        nc.vector.scalar_tensor_tensor(
            em32_t[b][:, 512 * c:512 * (c + 1)],
            mask_all[:, b * NP + 512 * c:b * NP + 512 * (c + 1)],
            -1e10, pen[:],
            op0=mybir.AluOpType.mult, op1=mybir.AluOpType.add,
        )

    # ------------- softmax row path (batches 0..BL-2) -------------
    def sm_row(b):
        em32 = em32_t.pop(b)
        negmax = p_sc.tile([1, 1], F32, tag="negmax", name=f"negmax{b}")
        nc.vector.reduce_max(negmax[:], em32[:], axis=AX.X, negate=True)
        em16 = p_em16.tile([1, NP], FP16, tag="em16", name=f"em16_{b}")
        zs = p_sc.tile([1, 1], F32, tag="zs", name=f"zs{b}")
        nc.scalar.activation(
            em16[:], em32[:], AF.Exp, bias=negmax[:], scale=1.0, accum_out=zs[:]
        )
        nc.gpsimd.dma_start(zs_out[b:b + 1, :], zs[:])
        em16_t[b] = em16

    pb_t = {}

    # ------------- pass 2 for batches 0..BL-2: DVE free-axis reduce -------
    def p2_bcast(b):
        # broadcast the weights row to all partitions: rank-1 PE matmul.
        # Emitted right after sm_row so the DVE ctx accumulates can overlap
        # the NEXT batch's matmul blocks instead of queueing behind them.
        em16 = em16_t.pop(b)
        pb = pp_b.tile([128, NP], F32, tag="pb", name=f"pb_{b}")
        for c in range(C):
            nc.tensor.matmul(
                pb[:, 512 * c:512 * (c + 1)],
                lhsT=ones16,
                rhs=em16[:, 512 * c:512 * (c + 1)],
                start=True, stop=True,
            )
        pb_t[b] = pb

    def p2_dve(b):
        hst_c = hst_t.pop(b)
        pb = pb_t.pop(b)
        ctxT = p_ctxT.tile([128, HK], F32, tag="ctxT", name=f"ctxT_{b}")
        hview = hst_c[:].rearrange("p (c k f) -> p c k f", c=C, k=HK, f=512)
        bview = pb[:].rearrange("p (c f) -> p c f", c=C, f=512)
        for m in range(HK):
            scr = p_scr.tile([128, NP], FP16, tag="scr", name=f"scr_{b}_{m}")
            nc.vector.scalar_tensor_tensor(
                out=scr[:].rearrange("p (c f) -> p c f", c=C, f=512),
                in0=hview[:, :, m, :],
                scalar=1.0,
                in1=bview,
                op0=mybir.AluOpType.mult, op1=mybir.AluOpType.mult,
                accum_out=ctxT[:, m:m + 1],
            )
        nc.gpsimd.dma_start(ctxT_out[b], ctxT[:])

    # ------------- pass 2 for the last batch: PE path -------------
    def p2_pe(b):
        em32 = em32_t.pop(b)
        hst_t.pop(b)
        # transpose energies to [s%128 partition, s//128]; the global max is
        # then a cheap per-partition max + gpsimd cross-partition all-reduce
        # (the [1,NP] row max would run on a single DVE lane).
        pt = pp_t.tile([128, TP], F32, tag="pt", name="ptT")
        for t in range(TP):
            nc.tensor.transpose(
                pt[:, t:t + 1], em32[:, 128 * t:128 * (t + 1)], ident32
            )
        pmax = p_sc.tile([128, 1], F32, tag="pmax")
        nc.vector.reduce_max(pmax[:], pt[:], axis=AX.X)
        gmax = p_sc.tile([128, 1], F32, tag="gmax")
        import concourse.bass_isa as bass_isa
        nc.gpsimd.partition_all_reduce(gmax[:], pmax[:], channels=128,
                                       reduce_op=bass_isa.ReduceOp.max)
        nmb = p_nm.tile([128, 1], F32, tag="nmb")
        nc.scalar.mul(nmb[:], gmax[:], -1.0)
        emt = p_emt.tile([128, TP], FP16, tag="emt")
        zsp = p_sc.tile([128, 1], F32, tag="zsp")
        nc.scalar.activation(
            emt[:], pt[:], AF.Exp, bias=nmb[:], scale=1.0, accum_out=zsp[:]
        )
        nc.gpsimd.dma_start(zsp_out[:], zsp[:])
        hsn_c = hsn_t[0]
        pc = [
            pp_en.tile([1, 512], F32, tag="pen", name=f"pctx{n}")
            for n in range(2)
        ]
        ctxr_sb = p_emt.tile([1, H], F32, tag="ctxr")
        for n in range(2):
            for t in range(TP):
                nc.tensor.matmul(
                    pc[n][:],
                    lhsT=emt[:, t:t + 1],
                    rhs=hsn_c[:, (n * TP + t) * 512:(n * TP + t + 1) * 512],
                    start=(t == 0), stop=(t == TP - 1),
                )
            nc.vector.tensor_copy(ctxr_sb[:, 512 * n:512 * (n + 1)], pc[n][:])
            nc.gpsimd.dma_start(ctxr_out[:, 512 * n:512 * (n + 1)],
                                ctxr_sb[:, 512 * n:512 * (n + 1)])

    hsn_t = {}

    def load_hsn():
        t = p_hsn.tile([128, TP * H], FP16, tag="hsn")
        nc.sync.dma_start(t[:], hsn[:])
        hsn_t[0] = t

    # ------------- schedule -------------
    if BL > 1:
        load_hst(1, nc.sync)
    for c in range(C):
        p1_block(0, c)
    for b in range(1, BL):
        if b + 1 < BL:
            load_hst(b + 1, nc.sync)
        if b == min(2, BL - 1):
            load_hsn()
        p1_block(b, 0)
        if b - 1 < BL - 1:
            sm_row(b - 1)
            p2_bcast(b - 1)
            # ctx accumulates queued BEFORE block(b,1)'s mask-stt: the DVE
            # runs them during the block instead of after it, and the ctxT
            # DMA no longer head-of-line-blocks the gpsimd queue at the tail.
            p2_dve(b - 1)
        for c in range(1, C):
            p1_block(b, c)
    if BL == 1:
        load_hsn()
    p2_pe(BL - 1)


def build_program(NP=1024):
    key = ("nc", NP)
    if key in _CACHE:
        return _CACHE[key]
    C = NP // 512
    TP = NP // 128
    nc = bacc.Bacc("TRN2", target_bir_lowering=False, debug=False, enable_asserts=False)
    aps = {
        "hst": nc.dram_tensor("hst", (BL, 128, C * HK * 512), FP16, kind="ExternalInput").ap(),
        "w1m": nc.dram_tensor("w1m", (128, HK * HK * 128), FP16, kind="ExternalInput").ap(),
        "qt": nc.dram_tensor("qt", (128, BL * HK), F32, kind="ExternalInput").ap(),
        "vt": nc.dram_tensor("vt", (128, HK), FP16, kind="ExternalInput").ap(),
        "cst": nc.dram_tensor("cst", (1, 130), FP16, kind="ExternalInput").ap(),
        "masku": nc.dram_tensor("masku", (1, BL * NP), U8, kind="ExternalInput").ap(),
        "hsn": nc.dram_tensor("hsn", (128, TP * H), FP16, kind="ExternalInput").ap(),
        "ctxT": nc.dram_tensor("ctxT", (BL, 128, HK), F32, kind="ExternalOutput").ap(),
        "ctxr": nc.dram_tensor("ctxr", (1, H), F32, kind="ExternalOutput").ap(),
        "zs": nc.dram_tensor("zs", (BL, 1), F32, kind="ExternalOutput").ap(),
        "zsp": nc.dram_tensor("zsp", (128, 1), F32, kind="ExternalOutput").ap(),
    }
    with tile.TileContext(nc) as tc:
        with ExitStack() as stack:
            aps["ctx_stack"] = stack
            _emit(tc, aps, NP)
    nc.compile()
    _CACHE[key] = nc
    return nc


def prep_in_maps(inputs):
    hs = np.asarray(inputs["hidden_sequence"], dtype=np.float32)
    hid = np.asarray(inputs["hidden"], dtype=np.float32)[0]  # (B, H)
    masks = np.asarray(inputs["input_masks"]).astype(bool)
    W1 = np.asarray(inputs["W1"], dtype=np.float32)
    W2 = np.asarray(inputs["W2"], dtype=np.float32)
    b1 = np.asarray(inputs["b1"], dtype=np.float32)
    b2 = np.asarray(inputs["b2"], dtype=np.float32)
    v = np.asarray(inputs["v"], dtype=np.float32)

    counts = masks.sum(axis=0)
    NP = max(512, int(-(-int(counts.max()) // 512)) * 512)
    C = NP // 512
    TP = NP // 128

    # w1m[p, (m*HK + k)*128 + j] = W1[128m + j, 128k + p]
    w1m = np.ascontiguousarray(
        W1.reshape(HK, 128, HK, 128).transpose(3, 0, 2, 1).reshape(128, HK * HK * 128)
    ).astype(np.float16)
    vt = np.ascontiguousarray(v.reshape(HK, 128).T).astype(np.float16)
    cst = np.zeros((1, 130), dtype=np.float16)
    cst[0, :128] = 1.0
    cst[0, 128:130] = np.frombuffer(
        np.float32(1.0).tobytes(), dtype=np.float16
    )
    # q[b, :] = W2 @ hidden[b] + b1 + b2 (host bias prep, S-independent)
    qfull = (hid.astype(np.float16).astype(np.float32)
             @ W2.astype(np.float16).astype(np.float32).T + b1 + b2)  # (B, H)

    in_maps = []
    for ci in range(NCORES):
        hstp = np.zeros((BL, 128, C * HK * 512), dtype=np.float16)
        hsnp = np.zeros((128, TP * H), dtype=np.float16)
        invm = np.ones((BL, NP), dtype=np.uint8)
        for bi in range(BL):
            b = BL * ci + bi
            idx = np.flatnonzero(masks[:, b])
            n = len(idx)
            hb = np.zeros((NP, H), dtype=np.float16)
            hb[:n] = hs[idx, b, :].astype(np.float16)  # compact (n, H)
            # hst[b, p, (c*HK + k)*512 + j] = hb[512c + j, 128k + p]
            hstp[bi] = (
                hb.reshape(C, 512, HK, 128).transpose(3, 0, 2, 1).reshape(128, C * HK * 512)
            )
            if bi == BL - 1:
                # hsn[p, (n*TP + t)*512 + j] = hb[128t + p, 512n + j]: the
                # last batch's ctx matmuls stream rhs contiguously in t.
                hsnp[:] = (
                    hb.reshape(TP, 128, 2, 512).transpose(1, 2, 0, 3)
                    .reshape(128, TP * H)
                )
            invm[bi, :n] = 0
        g = slice(BL * ci, BL * (ci + 1))
        # qt[p, BL*m + b] = q[b, 128m + p]
        qtp = np.ascontiguousarray(
            qfull[g].T.reshape(HK, 128, BL).transpose(1, 0, 2).reshape(128, HK * BL)
        )
        in_maps.append({
            "hst": hstp,
            "w1m": w1m,
            "qt": qtp,
            "vt": vt,
            "cst": cst,
            "masku": np.ascontiguousarray(invm.reshape(1, BL * NP)),
            "hsn": hsnp,
        })
    return in_maps, NP


def postprocess(results):
    """results[ci] -> dict with ctxT/ctxr/zs/zsp; returns (1,B,H) float32."""
    ctx = np.empty((B, H), dtype=np.float32)
    for ci in range(NCORES):
        r = results[ci]
        ctxT = np.asarray(r["ctxT"], dtype=np.float32)
        zs = np.asarray(r["zs"], dtype=np.float32)
        for bi in range(BL - 1):
            ctx[BL * ci + bi] = ctxT[bi].T.reshape(H) / zs[bi, 0]
        z_last = np.asarray(r["zsp"], dtype=np.float32).sum()
        ctx[BL * ci + BL - 1] = np.asarray(r["ctxr"], dtype=np.float32)[0] / z_last
    return ctx[None]


def kernel(**inputs):
    in_maps, NP = prep_in_maps(inputs)
    nc = build_program(NP)
    res = bass_utils.run_bass_kernel_spmd(nc, in_maps, list(range(NCORES)))
    return postprocess(res.results)


if __name__ == "__main__":
    build_program()
    print("program built OK")


# revision 21
# speedup vs baseline: 1.0485x; 1.0172x over previous
"""Bahdanau additive attention on TRN2, data-parallel over batch on 8 NeuronCores.

Reference computation (per batch b):
    pre[s, :]  = W1 @ hs[s, b, :] + b1 + W2 @ hidden[b, :] + b2      # (S, H)
    energy[s]  = v . tanh(pre[s, :])                                  # (S,)
    energy     = where(mask[s, b], energy, -1e10)
    attn       = softmax(energy over s)
    ctx[b, :]  = sum_s attn[s] * hs[s, b, :]                          # (H,)

Key optimizations over a dense kernel:
  - Mask compaction on the host: masked-out s positions contribute exactly
    zero attention (energy -1e10 -> exp 0), so only the unmasked positions
    are shipped/processed. Per-batch sequences are gathered to
    NP = roundup(max count, 512); pad columns carry mask=1.
  - fp16 matmul inputs (true 1 cycle/row on the PE; f32r measures ~1.3),
    fp32 PSUM accumulation. Measured max-rel-err ~2.4e-3 (gate 2e-2).
  - SBUF layouts keep every matmul's moving-data reads CONTIGUOUS: strided
    rhs jumps between back-to-back matmuls cost ~+50ns each (measured), so
    hst is stored [(c,k) blocks, 512] and tanh outputs go to a per-block
    ring [m*512] so the k/m loops stream sequentially.
  - q = W2 @ hidden + b1 + b2 is computed on the host (0.02% of FLOPs,
    S-independent bias prep) and uploaded as the tanh per-partition bias.
  - Context for batches 0..BL-2: exp weights row is broadcast to all 128
    partitions by one PE rank-1 matmul (ones x em16) into PSUM, then
    ctx[h-chunk] = sum_s hst[h,s]*w[s] is a DVE scalar_tensor_tensor
    free-axis accumulate per 128-row h-chunk (no second hs copy, no PE).
  - Context for the LAST batch runs on the then-idle PE instead (s-major
    hs copy + transposed-exp weights + 16 M=1 matmuls), cutting the
    end-of-kernel serial tail roughly in half.
  - Softmax is unnormalized on device; Z (or its per-partition partials)
    is exported and divided out on the host during unsharding.
"""

import sys
from contextlib import ExitStack

import numpy as np

# Fallback path for concourse; the axon sitecustomize normally provides it.
if "/opt/trn_rl_repo" not in sys.path:
    sys.path.append("/opt/trn_rl_repo")

import concourse.bass as bass
import concourse.bacc as bacc
import concourse.mybir as mybir
import concourse.tile as tile
from concourse import bass_utils

S, B, H = 2048, 32, 1024
NCORES = 8
BL = B // NCORES  # local batches per core
HK = H // 128     # 128-partition chunks of H

F32 = mybir.dt.float32
F32R = mybir.dt.float32r
FP16 = mybir.dt.float16
U8 = mybir.dt.uint8
AF = mybir.ActivationFunctionType
AX = mybir.AxisListType

_CACHE = {}


def _emit(tc, aps, NP):
    nc = tc.nc
    ctx = aps["ctx_stack"]
    C = NP // 512   # 512-wide sigma blocks
    TP = NP // 128  # 128-wide chunks (last-batch transposes / hsn tiles)
    hst, w1m, qt, vt, cst, masku, hsn = (
        aps["hst"], aps["w1m"], aps["qt"], aps["vt"], aps["cst"],
        aps["masku"], aps["hsn"],
    )
    ctxT_out, ctxr_out, zs_out, zsp_out = (
        aps["ctxT"], aps["ctxr"], aps["zs"], aps["zsp"],
    )

    def pool(name, bufs, space="SBUF"):
        return ctx.enter_context(tc.tile_pool(name=name, bufs=bufs, space=space))

    p_hst = pool("hst", 3)
    p_w1 = pool("w1", 1)
    p_small = pool("small", 1)
    p_mask = pool("mask", 1)
    p_th = pool("th", 2)
    p_em32 = pool("em32", 2)
    p_em16 = pool("em16", 2)
    p_emt = pool("emt", 1)
    p_scr = pool("scr", 2)
    p_ctxT = pool("ctxT", 2)
    p_sc = pool("sc", 8)
    p_nm = pool("nm", 1)
    p_hsn = pool("hsn", 1)

    pp_pre = pool("ppre", 3, space="PSUM")
    pp_en = pool("pen", 2, space="PSUM")
    pp_b = pool("pb", 1, space="PSUM")
    pp_t = pool("pt", 1, space="PSUM")

    # ---------------- setup DMAs ----------------
    # w1 m=0 chunk first (unblocks the very first matmuls), rest behind.
    w1_sb = p_w1.tile([128, HK * HK * 128], FP16, tag="w1")
    hst_t = {}

    def load_hst(b, queue, split=False):
        t = p_hst.tile([128, C * HK * 512], FP16, tag="hst", name=f"hst{b}")
        if split:
            for c in range(C):
                queue.dma_start(
                    t[:, c * HK * 512:(c + 1) * HK * 512],
                    hst[b, :, c * HK * 512:(c + 1) * HK * 512],
                )
        else:
            queue.dma_start(t[:], hst[b])
        hst_t[b] = t

    # priority order on one queue: the DMA engines drain a queue roughly
    # in order, so startup-critical bytes must precede prefetches.
    t0 = p_hst.tile([128, C * HK * 512], FP16, tag="hst", name="hst0")
    nc.sync.dma_start(t0[:, 0:HK * 256], hst[0, :, 0:HK * 256])
    nc.sync.dma_start(w1_sb[:, 0:HK * 128], w1m[:, 0:HK * 128])
    nc.sync.dma_start(t0[:, HK * 256:HK * 512], hst[0, :, HK * 256:HK * 512])
    hst_t[0] = t0
    nc.sync.dma_start(w1_sb[:, HK * 128:2 * HK * 128], w1m[:, HK * 128:2 * HK * 128])
    nc.sync.dma_start(w1_sb[:, 2 * HK * 128:4 * HK * 128], w1m[:, 2 * HK * 128:4 * HK * 128])
    nc.sync.dma_start(w1_sb[:, 4 * HK * 128:], w1m[:, 4 * HK * 128:])
    for c in range(1, C):
        nc.sync.dma_start(t0[:, c * HK * 512:(c + 1) * HK * 512],
                          hst[0, :, c * HK * 512:(c + 1) * HK * 512])

    # small constants on the vector queue (parallel issue)
    qt_sb = p_small.tile([128, BL * HK], F32, tag="qt")
    nc.scalar.dma_start(qt_sb[:], qt[:])
    vt_sb = p_small.tile([128, HK], FP16, tag="vt")
    nc.scalar.dma_start(vt_sb[:], vt[:])
    cst_sb = p_small.tile([1, 130], FP16, tag="cst")
    nc.scalar.dma_start(cst_sb[:], cst[:])
    ones16 = cst_sb[:, 0:128]
    ident32 = cst_sb[:, 128:130].bitcast(F32)
    mask_all = p_mask.tile([1, BL * NP], U8, tag="mask")
    nc.scalar.dma_start(mask_all[:], masku[:])

    em32_t = {}
    em16_t = {}
    nmrow_t = {}

    # ------------- pass 1: energies for one (batch, sigma-block) -------------
    def p1_block(b, c):
        if c == 0:
            em32_t[b] = p_em32.tile([1, NP], F32, tag="em32", name=f"em32_{b}")
        hst_c = hst_t[b]
        pen = pp_en.tile([1, 512], F32, tag="pen", name=f"pen_{b}_{c}")
        thr = p_th.tile([128, HK * 512], FP16, tag="th", name=f"th_{b}_{c}")
        for m in range(HK):
            ppre = pp_pre.tile([128, 512], F32, tag="ppre", name=f"ppre_{b}_{c}_{m}")
            for k in range(HK):
                nc.tensor.matmul(
                    ppre[:],
                    lhsT=w1_sb[:, (m * HK + k) * 128:(m * HK + k + 1) * 128],
                    rhs=hst_c[:, (c * HK + k) * 512:(c * HK + k + 1) * 512],
                    start=(k == 0), stop=(k == HK - 1),
                )
            nc.scalar.activation(
                thr[:, m * 512:(m + 1) * 512], ppre[:], AF.Tanh,
                bias=qt_sb[:, BL * m + b:BL * m + b + 1], scale=1.0,
            )
        # energy matmuls as one sequential run over the thr ring; on-PE v-dot
        # reads th exactly once with no extra SBUF traffic (a DVE FMA tree
        # measured SLOWER overall: its acc read+write traffic contends with
        # the PE rhs stream and tanh writes, slowing both by ~20%).
        for m in range(HK):
            nc.tensor.matmul(
                pen[:], lhsT=vt_sb[:, m:m + 1],
                rhs=thr[:, m * 512:(m + 1) * 512],
                start=(m == 0), stop=(m == HK - 1),
            )
        # mask + PSUM drain in one DVE op: em = minv * -1e10 + energy
        nc.vector.scalar_tensor_tensor(
            em32_t[b][:, 512 * c:512 * (c + 1)],
            mask_all[:, b * NP + 512 * c:b * NP + 512 * (c + 1)],
            -1e10, pen[:],
            op0=mybir.AluOpType.mult, op1=mybir.AluOpType.add,
        )
        if b == BL - 1:
            # last batch: per-block -max now (hidden under the blocks) so the
            # tail only reduces C scalars instead of a [1,NP] row.
            if c == 0:
                nmrow_t[b] = p_sc.tile([1, C], F32, tag="nmrow", name="nmrow")
            nc.vector.reduce_max(
                nmrow_t[b][:, c:c + 1], em32_t[b][:, 512 * c:512 * (c + 1)],
                axis=AX.X, negate=True,
            )

    # ------------- softmax row path (batches 0..BL-2) -------------
    def sm_row(b):
        em32 = em32_t.pop(b)
        negmax = p_sc.tile([1, 1], F32, tag="negmax", name=f"negmax{b}")
        nc.vector.reduce_max(negmax[:], em32[:], axis=AX.X, negate=True)
        em16 = p_em16.tile([1, NP], FP16, tag="em16", name=f"em16_{b}")
        zs = p_sc.tile([1, 1], F32, tag="zs", name=f"zs{b}")
        nc.scalar.activation(
            em16[:], em32[:], AF.Exp, bias=negmax[:], scale=1.0, accum_out=zs[:]
        )
        nc.gpsimd.dma_start(zs_out[b:b + 1, :], zs[:])
        em16_t[b] = em16

    pb_t = {}

    # ------------- pass 2 for batches 0..BL-2: DVE free-axis reduce -------
    def p2_bcast(b):
        # broadcast the weights row to all partitions: rank-1 PE matmul.
        # Emitted right after sm_row so the DVE ctx accumulates can overlap
        # the NEXT batch's matmul blocks instead of queueing behind them.
        em16 = em16_t.pop(b)
        pb = pp_b.tile([128, NP], F32, tag="pb", name=f"pb_{b}")
        for c in range(C):
            nc.tensor.matmul(
                pb[:, 512 * c:512 * (c + 1)],
                lhsT=ones16,
                rhs=em16[:, 512 * c:512 * (c + 1)],
                start=True, stop=True,
            )
        pb_t[b] = pb

    def p2_dve(b):
        hst_c = hst_t.pop(b)
        pb = pb_t.pop(b)
        ctxT = p_ctxT.tile([128, HK], F32, tag="ctxT", name=f"ctxT_{b}")
        hview = hst_c[:].rearrange("p (c k f) -> p c k f", c=C, k=HK, f=512)
        bview = pb[:].rearrange("p (c f) -> p c f", c=C, f=512)
        for m in range(HK):
            scr = p_scr.tile([128, NP], FP16, tag="scr", name=f"scr_{b}_{m}")
            nc.vector.scalar_tensor_tensor(
                out=scr[:].rearrange("p (c f) -> p c f", c=C, f=512),
                in0=hview[:, :, m, :],
                scalar=1.0,
                in1=bview,
                op0=mybir.AluOpType.mult, op1=mybir.AluOpType.mult,
                accum_out=ctxT[:, m:m + 1],
            )
        nc.gpsimd.dma_start(ctxT_out[b], ctxT[:])

    # ------------- pass 2 for the last batch: PE path -------------
    def p2_pe(b):
        em32 = em32_t.pop(b)
        hst_t.pop(b)
        # transpose energies to [s%128 partition, s//128]
        pt = pp_t.tile([128, TP], F32, tag="pt", name="ptT")
        for t in range(TP):
            nc.tensor.transpose(
                pt[:, t:t + 1], em32[:, 128 * t:128 * (t + 1)], ident32
            )
        # global -max: combine the C per-block partials (computed during the
        # blocks), broadcast to 128 partitions with a rank-1 PE matmul (the
        # gpsimd all-reduce costs ~1.3us of queue-wakeup latency here).
        negmax16 = p_sc.tile([1, 1], FP16, tag="negmax16")
        nc.vector.tensor_reduce(
            negmax16[:], nmrow_t.pop(b)[:], axis=AX.X, op=mybir.AluOpType.min
        )
        pbn = pp_b.tile([128, NP], F32, tag="pb", name="pb_nm")
        nc.tensor.matmul(
            pbn[:, 0:1], lhsT=ones16, rhs=negmax16[:], start=True, stop=True,
        )
        nmb = p_nm.tile([128, 1], F32, tag="nmb")
        nc.vector.tensor_copy(nmb[:], pbn[:, 0:1])
        emt = p_emt.tile([128, TP], FP16, tag="emt")
        zsp = p_sc.tile([128, 1], F32, tag="zsp")
        nc.scalar.activation(
            emt[:], pt[:], AF.Exp, bias=nmb[:], scale=1.0, accum_out=zsp[:]
        )
        nc.gpsimd.dma_start(zsp_out[:], zsp[:])
        hsn_c = hsn_t[0]
        pc = [
            pp_en.tile([1, 512], F32, tag="pen", name=f"pctx{n}")
            for n in range(2)
        ]
        ctxr_sb = p_emt.tile([1, H], F32, tag="ctxr")
        for n in range(2):
            for t in range(TP):
                nc.tensor.matmul(
                    pc[n][:],
                    lhsT=emt[:, t:t + 1],
                    rhs=hsn_c[:, (n * TP + t) * 512:(n * TP + t + 1) * 512],
                    start=(t == 0), stop=(t == TP - 1),
                )
            nc.vector.tensor_copy(ctxr_sb[:, 512 * n:512 * (n + 1)], pc[n][:])
            nc.gpsimd.dma_start(ctxr_out[:, 512 * n:512 * (n + 1)],
                                ctxr_sb[:, 512 * n:512 * (n + 1)])

    hsn_t = {}

    def load_hsn():
        t = p_hsn.tile([128, TP * H], FP16, tag="hsn")
        nc.sync.dma_start(t[:], hsn[:])
        hsn_t[0] = t

    # ------------- schedule -------------
    if BL > 1:
        load_hst(1, nc.sync)
    for c in range(C):
        p1_block(0, c)
    for b in range(1, BL):
        if b + 1 < BL:
            load_hst(b + 1, nc.sync)
        if b == min(2, BL - 1):
            load_hsn()
        p1_block(b, 0)
        if b - 1 < BL - 1:
            sm_row(b - 1)
            p2_bcast(b - 1)
            # ctx accumulates queued BEFORE block(b,1)'s mask-stt: the DVE
            # runs them during the block instead of after it, and the ctxT
            # DMA no longer head-of-line-blocks the gpsimd queue at the tail.
            p2_dve(b - 1)
        for c in range(1, C):
            p1_block(b, c)
    if BL == 1:
        load_hsn()
    p2_pe(BL - 1)


def build_program(NP=1024):
    key = ("nc", NP)
    if key in _CACHE:
        return _CACHE[key]
    C = NP // 512
    TP = NP // 128
    nc = bacc.Bacc("TRN2", target_bir_lowering=False, debug=False, enable_asserts=False)
    aps = {
        "hst": nc.dram_tensor("hst", (BL, 128, C * HK * 512), FP16, kind="ExternalInput").ap(),
        "w1m": nc.dram_tensor("w1m", (128, HK * HK * 128), FP16, kind="ExternalInput").ap(),
        "qt": nc.dram_tensor("qt", (128, BL * HK), F32, kind="ExternalInput").ap(),
        "vt": nc.dram_tensor("vt", (128, HK), FP16, kind="ExternalInput").ap(),
        "cst": nc.dram_tensor("cst", (1, 130), FP16, kind="ExternalInput").ap(),
        "masku": nc.dram_tensor("masku", (1, BL * NP), U8, kind="ExternalInput").ap(),
        "hsn": nc.dram_tensor("hsn", (128, TP * H), FP16, kind="ExternalInput").ap(),
        "ctxT": nc.dram_tensor("ctxT", (BL, 128, HK), F32, kind="ExternalOutput").ap(),
        "ctxr": nc.dram_tensor("ctxr", (1, H), F32, kind="ExternalOutput").ap(),
        "zs": nc.dram_tensor("zs", (BL, 1), F32, kind="ExternalOutput").ap(),
        "zsp": nc.dram_tensor("zsp", (128, 1), F32, kind="ExternalOutput").ap(),
    }
    with tile.TileContext(nc) as tc:
        with ExitStack() as stack:
            aps["ctx_stack"] = stack
            _emit(tc, aps, NP)
    nc.compile()
    _CACHE[key] = nc
    return nc


def prep_in_maps(inputs):
    hs = np.asarray(inputs["hidden_sequence"], dtype=np.float32)
    hid = np.asarray(inputs["hidden"], dtype=np.float32)[0]  # (B, H)
    masks = np.asarray(inputs["input_masks"]).astype(bool)
    W1 = np.asarray(inputs["W1"], dtype=np.float32)
    W2 = np.asarray(inputs["W2"], dtype=np.float32)
    b1 = np.asarray(inputs["b1"], dtype=np.float32)
    b2 = np.asarray(inputs["b2"], dtype=np.float32)
    v = np.asarray(inputs["v"], dtype=np.float32)

    counts = masks.sum(axis=0)
    NP = max(512, int(-(-int(counts.max()) // 512)) * 512)
    C = NP // 512
    TP = NP // 128

    # w1m[p, (m*HK + k)*128 + j] = W1[128m + j, 128k + p]
    w1m = np.ascontiguousarray(
        W1.reshape(HK, 128, HK, 128).transpose(3, 0, 2, 1).reshape(128, HK * HK * 128)
    ).astype(np.float16)
    vt = np.ascontiguousarray(v.reshape(HK, 128).T).astype(np.float16)
    cst = np.zeros((1, 130), dtype=np.float16)
    cst[0, :128] = 1.0
    cst[0, 128:130] = np.frombuffer(
        np.float32(1.0).tobytes(), dtype=np.float16
    )
    # q[b, :] = W2 @ hidden[b] + b1 + b2 (host bias prep, S-independent)
    qfull = (hid.astype(np.float16).astype(np.float32)
             @ W2.astype(np.float16).astype(np.float32).T + b1 + b2)  # (B, H)

    in_maps = []
    for ci in range(NCORES):
        hstp = np.zeros((BL, 128, C * HK * 512), dtype=np.float16)
        hsnp = np.zeros((128, TP * H), dtype=np.float16)
        invm = np.ones((BL, NP), dtype=np.uint8)
        for bi in range(BL):
            b = BL * ci + bi
            idx = np.flatnonzero(masks[:, b])
            n = len(idx)
            hb = np.zeros((NP, H), dtype=np.float16)
            hb[:n] = hs[idx, b, :].astype(np.float16)  # compact (n, H)
            # hst[b, p, (c*HK + k)*512 + j] = hb[512c + j, 128k + p]
            hstp[bi] = (
                hb.reshape(C, 512, HK, 128).transpose(3, 0, 2, 1).reshape(128, C * HK * 512)
            )
            if bi == BL - 1:
                # hsn[p, (n*TP + t)*512 + j] = hb[128t + p, 512n + j]: the
                # last batch's ctx matmuls stream rhs contiguously in t.
                hsnp[:] = (
                    hb.reshape(TP, 128, 2, 512).transpose(1, 2, 0, 3)
                    .reshape(128, TP * H)
                )
            invm[bi, :n] = 0
        g = slice(BL * ci, BL * (ci + 1))
        # qt[p, BL*m + b] = q[b, 128m + p]
        qtp = np.ascontiguousarray(
            qfull[g].T.reshape(HK, 128, BL).transpose(1, 0, 2).reshape(128, HK * BL)
        )
        in_maps.append({
            "hst": hstp,
            "w1m": w1m,
            "qt": qtp,
            "vt": vt,
            "cst": cst,
            "masku": np.ascontiguousarray(invm.reshape(1, BL * NP)),
            "hsn": hsnp,
        })
    return in_maps, NP


def postprocess(results):
    """results[ci] -> dict with ctxT/ctxr/zs/zsp; returns (1,B,H) float32."""
    ctx = np.empty((B, H), dtype=np.float32)
    for ci in range(NCORES):
        r = results[ci]
        ctxT = np.asarray(r["ctxT"], dtype=np.float32)
        zs = np.asarray(r["zs"], dtype=np.float32)
        for bi in range(BL - 1):
            ctx[BL * ci + bi] = ctxT[bi].T.reshape(H) / zs[bi, 0]
        z_last = np.asarray(r["zsp"], dtype=np.float32).sum()
        ctx[BL * ci + BL - 1] = np.asarray(r["ctxr"], dtype=np.float32)[0] / z_last
    return ctx[None]


def kernel(**inputs):
    in_maps, NP = prep_in_maps(inputs)
    nc = build_program(NP)
    res = bass_utils.run_bass_kernel_spmd(nc, in_maps, list(range(NCORES)))
    return postprocess(res.results)


if __name__ == "__main__":
    build_program()
    print("program built OK")
